# revision 18
# baseline (speedup 1.0000x reference)
"""Trainium2 Bass kernel for modulated multi-head attention (q=k=v variant).

v4 — fp8 DoubleRow attention with column-biased exp:

  1. attnv in fp8e4 DoubleRow (K=256 per matmul, 2x PE throughput):
     stationary v [128, 2, 64] fp8, moving fp8 E tile [128, 2, 512].
     Precision recovered by a value split v = Q(v) + Q(v - Q(v)); both
     matmul streams accumulate into the same PSUM group.
  2. Rowsums via an extra DoubleRow matmul with an all-ones stationary:
     its [64, 512] output replicates Sum_m E[m, n] on every partition, so
     normalization is reciprocal (PSUM->SBUF) + multiply - no broadcast.
  3. exp range control for fp8: E~[m,n] = exp((s[m,n] + w[n])/8) with
     w[n] = 8c - |q_n|^2.  The shift is applied per OUTPUT token n (the
     free dim), so it cancels exactly in the ones-rowsum normalization;
     empirically s[m,n] <= |q_n|^2 (q=k), so E~ <= e^c = 90 < 240 (fp8e4
     max).  The shift rides as a 65th contraction row of the scores
     matmul: stationary qstat row 64 = ones, moving qmov row 64 = w[n]
     (built by a tiny per-pair DMA transpose of the DVE-reduced |q|^2).
     The Scalar exp has no bias read and writes fp8 directly.

  Engine budget per core @2.4GHz: PE ~180k cycles (~75us), Scalar 64 exps
  (~68us), DVE ~45us, Pool ~12us.  Per head-phase, scores(h) batches
  interleave with attnv(h-1) and yproj fillers so the Scalar exp stream
  never starves and the PE keeps high duty for p-state.

Sharding: data-parallel over batch B=8, one batch element per NeuronCore.
"""

import sys

if "/opt/trn_rl_repo" not in sys.path:
    sys.path.insert(0, "/opt/trn_rl_repo")

from contextlib import ExitStack

import numpy as np

import concourse.bass as bass
import concourse.bacc as bacc
import concourse.mybir as mybir
import concourse.tile as tile
from concourse.bass_utils import run_bass_kernel_spmd

P = 128          # partitions
F = 512          # hidden dim
C4 = F // P      # 4 feature chunks of 128
N = 1024         # tokens
NB = N // P      # 8 token blocks
H = 8            # heads
D = 64           # head dim
B = 8            # batch (one per core)
SCALE = 1.0 / 8.0
CBIAS = 4.5      # exp range shift: E in (0, ~90] inside fp8e4's [2^-10, 240]
EPS = 1e-8

F32 = mybir.dt.float32
BF16 = mybir.dt.bfloat16
FP8 = mybir.dt.float8e4


def _split_free(apnd, group, inner):
    """View a [..., group*inner] stride-1 AP as [..., group, inner]."""
    assert apnd.ap[-1][0] == 1 and apnd.ap[-1][1] == group * inner, apnd.ap
    return bass.AP(
        tensor=apnd.tensor,
        offset=apnd.offset,
        ap=[list(d) for d in apnd.ap[:-1]] + [[inner, group], [1, inner]],
    )


def _ap(t, offset, dims):
    """Raw AP into a tile at an element offset."""
    return bass.AP(tensor=t.tensor, offset=t.offset + offset,
                   ap=[list(d) for d in dims])


def _emit(nc, loop_reps=0):
    xT = nc.dram_tensor("xT", [F, N], BF16, kind="ExternalInput")
    wkT = nc.dram_tensor("wkT", [F, F], BF16, kind="ExternalInput")
    woT = nc.dram_tensor("woT", [F, F], BF16, kind="ExternalInput")
    y = nc.dram_tensor("y", [N, F], F32, kind="ExternalOutput")

    with tile.TileContext(nc) as tc:
        if loop_reps:
            with tc.For_i(0, loop_reps, 1):
                _emit_body(nc, tc, xT, wkT, woT, y)
        else:
            _emit_body(nc, tc, xT, wkT, woT, y)


def _emit_body(nc, tc, xT, wkT, woT, y):
    f32 = F32
    Exp = mybir.ActivationFunctionType.Exp
    Square = mybir.ActivationFunctionType.Square
    Copy = mybir.ActivationFunctionType.Copy
    MULT = mybir.AluOpType.mult
    SUB = mybir.AluOpType.subtract
    ADD = mybir.AluOpType.add
    DR = mybir.MatmulPerfMode.DoubleRow
    AX = mybir.AxisListType.X

    with ExitStack() as ctx:
        persist = ctx.enter_context(tc.tile_pool(name="persist", bufs=1))
        dram = ctx.enter_context(tc.tile_pool(name="dram", bufs=4, space="DRAM"))
        psum = ctx.enter_context(tc.tile_pool(name="psum", bufs=1, space="PSUM"))
        att = ctx.enter_context(tc.tile_pool(name="att", bufs=1))
        attrs = ctx.enter_context(tc.tile_pool(name="attrs", bufs=2))

        # ---- persistent SBUF tiles ----
        xT_sb = persist.tile([P, C4, N], BF16)
        wk_sb = persist.tile([P, C4, F], BF16)
        wo_sb = persist.tile([D, H, F], BF16)
        qstat = persist.tile([D + 1, H, N], BF16)  # q rows + ones row (stat)
        qmov = persist.tile([D + 1, H, N], BF16)   # q rows + w row (moving)
        kqv_bf = persist.tile([P, NB, F], BF16)    # v bf16, [tok, feat]
        v8 = persist.tile([P, NB, F], FP8)         # v fp8
        dv8 = persist.tile([P, NB, F], FP8)        # v - Q(v)
        ones8 = persist.tile([P, 2, D], FP8)       # DoubleRow rowsum stationary
        sqs = persist.tile([P, NB, F], BF16)       # q^2 scratch
        negf = persist.tile([P, NB, H], f32)       # sumsq (f32 reduce out)
        negb = persist.tile([P, NB, H], BF16)      # w = 8c - sumsq
        aT = persist.tile([D, H, N], BF16)         # normalized attn outT
        y_acc = persist.tile([P, NB, F], f32)

        # exp-table prewarm while input DMAs stream
        warm = persist.tile([1, 1], f32)
        nc.vector.memset(warm, 1.0)
        nc.scalar.activation(out=warm, in_=warm, func=Exp, scale=1.0)

        # ---- input DMAs: token-quarters so pair 0 starts early ----
        wk_r = wkT.rearrange("(c p) o -> p c o", p=P)
        nc.gpsimd.dma_start(out=wk_sb[:, 0:2, :], in_=wk_r[:, 0:2, :])
        nc.sync.dma_start(out=wk_sb[:, 2:4, :], in_=wk_r[:, 2:4, :])
        xT_r = xT.rearrange("(c p) n -> p c n", p=P)
        for quar in range(4):
            sl = slice(quar * 256, (quar + 1) * 256)
            eng = nc.sync if quar % 2 == 0 else nc.scalar
            eng.dma_start(out=xT_sb[:, :, sl], in_=xT_r[:, :, sl])
        nc.gpsimd.dma_start(out=wo_sb, in_=woT.rearrange("(h d) o -> d h o", d=D))

        nc.vector.memset(ones8, 1.0)
        nc.gpsimd.memset(qstat[D : D + 1, :, :], 1.0)

        def sc_tile():
            return psum.tile([P, N], f32, tag="sc", bufs=2, name="sc")

        # ---- kqv_v pair: token blocks 2p, 2p+1 -> [tok, F] ----
        def emit_kqv_v_pair(p):
            pt = sc_tile()
            for c in range(C4):
                for k in range(2):
                    nc.tensor.matmul(
                        pt[:, k * F : (k + 1) * F],
                        xT_sb[:, c, (2 * p + k) * P : (2 * p + k + 1) * P],
                        wk_sb[:, c, :],
                        start=(c == 0), stop=(c == C4 - 1))
            return pt

        def emit_v_evac(p, pt):
            """negb chain: Scalar squares (idle pre-exp), DVE reduce -> w,
            then a tiny DMA transpose into qmov row 64. Pool does fp8 v."""
            sl2 = slice(2 * p, 2 * p + 2)
            nc.vector.tensor_copy(out=kqv_bf[:, sl2, :], in_=pt)
            nc.scalar.activation(out=sqs[:, sl2, :], in_=pt, func=Square)
            nc.vector.tensor_reduce(
                out=negf[:, sl2, :],
                in_=_split_free(sqs[:, sl2, :], H, D),
                axis=AX, op=ADD)
            nc.vector.tensor_scalar(
                out=negb[:, sl2, :], in0=negf[:, sl2, :],
                scalar1=-1.0, scalar2=8.0 * CBIAS, op0=MULT, op1=ADD)
            # DMA transpose: negb[tok-part, H] -> dram -> qmov row 64
            w_d = dram.tile([2 * H * P], BF16, tag="w_d", bufs=4, name="w_d")
            for k in range(2):
                nc.sync.dma_start(
                    out=_ap(w_d, k * P, [[1, P], [2 * P, H]]),
                    in_=negb[:, 2 * p + k, :])
            nc.gpsimd.dma_start(
                out=qmov[D : D + 1, :, 2 * p * P : (2 * p + 2) * P],
                in_=_ap(w_d, 0, [[2 * H * P, 1], [2 * P, H], [1, 2 * P]]))
            # fp8 v + residual (Pool reads SBUF only)
            nc.gpsimd.tensor_copy(out=v8[:, sl2, :], in_=kqv_bf[:, sl2, :])
            nc.gpsimd.tensor_tensor(
                out=dv8[:, sl2, :], in0=kqv_bf[:, sl2, :], in1=v8[:, sl2, :],
                op=SUB)

        # ---- kqvT chunk ob (heads 2ob, 2ob+1) -> qstat/qmov q rows ----
        def emit_kqvT_chunk(ob):
            pt = sc_tile()
            for c in range(C4):
                for nh in range(2):
                    nc.tensor.matmul(
                        pt[:, nh * F : (nh + 1) * F],
                        wk_sb[:, c, ob * P : (ob + 1) * P],
                        xT_sb[:, c, nh * F : (nh + 1) * F],
                        start=(c == 0), stop=(c == C4 - 1))
            # evac per head into qstat rows 0:64 (DVE handles the base-64
            # move for the odd head); h0 via Scalar (idle pre-exp)
            for hh in range(2):
                h = 2 * ob + hh
                src = pt[hh * D : (hh + 1) * D, :]
                if h == 0:
                    nc.scalar.activation(out=qstat[0:D, h, :], in_=src,
                                         func=Copy)
                else:
                    nc.vector.tensor_copy(out=qstat[0:D, h, :], in_=src)
                eng = nc.sync if h % 2 == 0 else nc.gpsimd
                eng.dma_start(out=qmov[0:D, h, :], in_=qstat[0:D, h, :])

        head_state = {}

        # ---- scores tiles + exp for head h (K=65: q rows + ones*w row) ----
        def emit_scores(h, mbs):
            E = head_state[h]["E"]
            for mb in mbs:
                s = sc_tile()
                for nh in range(2):
                    nc.tensor.matmul(
                        s[:, nh * F : (nh + 1) * F],
                        qstat[:, h, mb * P : (mb + 1) * P],
                        qmov[:, h, nh * F : (nh + 1) * F],
                        start=True, stop=True)
                nc.scalar.activation(out=E[:, mb, :], in_=s, func=Exp,
                                     scale=SCALE)

        def new_head(h):
            head_state[h] = dict(
                E=att.tile([P, NB, N], FP8, tag="E", bufs=3, name="E"))

        # ---- attnv for head h: fp8 DoubleRow; v8+dv8 one group into acc,
        # all-ones stationary into rs (rowsums replicated on 64 rows) ----
        def emit_attnv(h, kts):
            st = head_state[h]
            if "acc" not in st:
                st["acc"] = psum.tile([D, 2, F], f32, tag="acc", name="acc")
                st["rs"] = psum.tile([D, 2, F], f32, tag="rs", name="rs")
            acc, rs = st["acc"], st["rs"]
            E = st["E"]
            for kt in kts:
                for w8, is_one in ((v8, False), (dv8, False), (ones8, True)):
                    for nh in range(2):
                        out = (rs if is_one else acc)[:, nh, :]
                        lhsT = (w8[:, :, :] if is_one
                                else w8[:, 2 * kt : 2 * kt + 2,
                                        h * D : (h + 1) * D])
                        nc.tensor.matmul(
                            out, lhsT,
                            E[:, 2 * kt : 2 * kt + 2, nh * F : (nh + 1) * F],
                            start=(kt == kts[0] if is_one
                                   else (kt == 0 and w8 is v8)),
                            stop=(kt == kts[-1] if is_one
                                  else (kt == 3 and w8 is dv8)),
                            perf_mode=DR)

        # ---- reciprocal of rowsums: rs psum -> SBUF ----
        def emit_recip(h):
            st = head_state[h]
            rs_sb = attrs.tile([D, N], f32, tag="rs_sb", name="rs_sb")
            nc.vector.reciprocal(out=_split_free(rs_sb[:, :], 2, F),
                                 in_=st["rs"][:, :, :])
            st["rs_sb"] = rs_sb

        # ---- evac: aT[, h, ] = acc * recip_rowsums ----
        def emit_evac(h):
            st = head_state[h]
            nc.vector.tensor_tensor(
                out=_split_free(aT[:, h, :], 2, F),
                in0=st["acc"][:, :, :],
                in1=_split_free(st["rs_sb"][:, :], 2, F),
                op=MULT)

        # ---- yproj partial for head pair pc over 2-block tiles ----
        def emit_ypartial(pc, nbs, heads=(0, 1), with_dma=False, add=None):
            first_pair = (not add) if add is not None else (pc == 0)
            for i in range(0, len(nbs), 2):
                pt = sc_tile()
                for hh in heads:
                    h = 2 * pc + hh
                    for k in range(2):
                        nb = nbs[i + k]
                        nc.tensor.matmul(
                            pt[:, k * F : (k + 1) * F],
                            aT[:, h, nb * P : (nb + 1) * P],
                            wo_sb[:, h, :],
                            start=(hh == heads[0]), stop=(hh == heads[-1]))
                nb0 = nbs[i]
                if first_pair:
                    nc.vector.tensor_copy(out=y_acc[:, nb0 : nb0 + 2, :], in_=pt)
                else:
                    nc.vector.tensor_add(out=y_acc[:, nb0 : nb0 + 2, :],
                                         in0=y_acc[:, nb0 : nb0 + 2, :], in1=pt)
                if with_dma:
                    eng = nc.sync if nb0 % 4 == 0 else nc.gpsimd
                    eng.dma_start(
                        out=y.rearrange("(b p) f -> p b f", p=P)[:, nb0 : nb0 + 2, :],
                        in_=y_acc[:, nb0 : nb0 + 2, :])

        # ========== prologue: projections, w rows, heads 0-1 scores ==========
        for p in range(4):
            emit_v_evac(p, emit_kqv_v_pair(p))
        new_head(0)
        emit_kqvT_chunk(0)
        emit_scores(0, [0, 1])
        emit_kqvT_chunk(1)
        emit_scores(0, [2, 3])
        emit_kqvT_chunk(2)
        emit_scores(0, [4, 5])
        emit_kqvT_chunk(3)
        emit_scores(0, [6, 7])
        new_head(1)
        emit_scores(1, [0, 1, 2, 3])
        emit_scores(1, [4, 5, 6, 7])

        # ========== phase 2: catch up attnv(0) and attnv(1) ==========
        new_head(2)
        emit_scores(2, [0, 1])
        emit_attnv(0, [0, 1, 2, 3])
        emit_scores(2, [2, 3])
        emit_recip(0)
        emit_evac(0)
        emit_scores(2, [4, 5])
        emit_attnv(1, [0, 1, 2, 3])
        emit_scores(2, [6, 7])
        emit_recip(1)
        emit_evac(1)

        # ========== steady phases 3-7 ==========
        for h in range(3, H):
            new_head(h)
            emit_scores(h, [0, 1])
            emit_attnv(h - 1, [0, 1, 2, 3])
            emit_scores(h, [2, 3])
            emit_recip(h - 1)
            # filler batches (ypartial pc needs evac(2pc+1))
            if h == 3:
                emit_ypartial(0, [0, 1, 2, 3])
            elif h == 4:
                emit_ypartial(0, [4, 5, 6, 7])
            elif h == 5:
                emit_ypartial(1, [0, 1, 2, 3])
            elif h == 6:
                emit_ypartial(1, [4, 5, 6, 7])
            elif h == 7:
                emit_ypartial(2, list(range(NB)))
            emit_scores(h, [4, 5])
            emit_evac(h - 1)
            emit_scores(h, [6, 7])

        # ========== tail ==========
        h = H - 1
        emit_attnv(h, [0, 1, 2, 3])
        emit_recip(h)
        # h6 half of the final pair while h7's recip/evac run
        emit_ypartial(3, list(range(NB)), heads=(0,), add=True)
        emit_evac(h)
        emit_ypartial(3, list(range(NB)), heads=(1,), add=True, with_dma=True)


_NC_CACHE = None


def build_nc():
    global _NC_CACHE
    if _NC_CACHE is None:
        nc = bacc.Bacc(trn_type="TRN2")
        _emit(nc)
        nc.finalize()
        _NC_CACHE = nc
    return _NC_CACHE


def _eff_weightT(weight, style):
    """Host: modulated+demodulated weight, transposed, bf16.
    weight [O, I] fp32, style [I] fp32 -> [I, O] bf16."""
    import ml_dtypes
    w = weight * style[None, :]
    w = w * (1.0 / np.sqrt((w * w).sum(axis=1) + EPS))[:, None]
    return np.ascontiguousarray(w.T.astype(ml_dtypes.bfloat16))


def make_in_maps(x, s, k_weight, k_aff_w, k_aff_b, o_weight, o_aff_w, o_aff_b):
    import ml_dtypes
    f = np.float32
    bf = ml_dtypes.bfloat16
    x = np.asarray(x, f)
    s = np.asarray(s, f)
    k_weight = np.asarray(k_weight, f)
    k_aff_w = np.asarray(k_aff_w, f)
    k_aff_b = np.asarray(k_aff_b, f)
    o_weight = np.asarray(o_weight, f)
    o_aff_w = np.asarray(o_aff_w, f)
    o_aff_b = np.asarray(o_aff_b, f)
    in_maps = []
    for b in range(B):
        style_k = s[b] @ k_aff_w.T + k_aff_b
        style_o = s[b] @ o_aff_w.T + o_aff_b
        in_maps.append({
            "xT": np.ascontiguousarray(x[b].T.astype(bf)),
            "wkT": _eff_weightT(k_weight, style_k),
            "woT": _eff_weightT(o_weight, style_o),
        })
    return in_maps


def kernel(x, s, k_weight, k_aff_w, k_aff_b, o_weight, o_aff_w, o_aff_b):
    assert x.shape == (B, N, F), x.shape
    nc = build_nc()
    in_maps = make_in_maps(x, s, k_weight, k_aff_w, k_aff_b,
                           o_weight, o_aff_w, o_aff_b)
    res = run_bass_kernel_spmd(nc, in_maps, list(range(B)))
    return np.stack([res.results[b]["y"] for b in range(B)], axis=0)


# revision 22
# speedup vs baseline: 1.7109x; 1.7109x over previous
"""Trainium2 Bass kernel for modulated multi-head attention (q=k=v variant).

v4 — fp8 DoubleRow attention with column-biased exp:

  1. attnv in fp8e4 DoubleRow (K=256 per matmul, 2x PE throughput):
     stationary v [128, 2, 64] fp8, moving fp8 E tile [128, 2, 512].
     Precision recovered by a value split v = Q(v) + Q(v - Q(v)); both
     matmul streams accumulate into the same PSUM group.
  2. Rowsums via an extra DoubleRow matmul with an all-ones stationary:
     its [64, 512] output replicates Sum_m E[m, n] on every partition, so
     normalization is reciprocal (PSUM->SBUF) + multiply - no broadcast.
  3. exp range control for fp8: E~[m,n] = exp((s[m,n] + w[n])/8) with
     w[n] = 8c - |q_n|^2.  The shift is applied per OUTPUT token n (the
     free dim), so it cancels exactly in the ones-rowsum normalization;
     empirically s[m,n] <= |q_n|^2 (q=k), so E~ <= e^c = 90 < 240 (fp8e4
     max).  The shift rides as a 65th contraction row of the scores
     matmul: stationary qstat row 64 = ones, moving qmov row 64 = w[n]
     (built by a tiny per-pair DMA transpose of the DVE-reduced |q|^2).
     The Scalar exp has no bias read and writes fp8 directly.

  Engine budget per core @2.4GHz: PE ~180k cycles (~75us), Scalar 64 exps
  (~68us), DVE ~45us, Pool ~12us.  Per head-phase, scores(h) batches
  interleave with attnv(h-1) and yproj fillers so the Scalar exp stream
  never starves and the PE keeps high duty for p-state.

Sharding: data-parallel over batch B=8, one batch element per NeuronCore.
"""

import sys

if "/opt/trn_rl_repo" not in sys.path:
    sys.path.insert(0, "/opt/trn_rl_repo")

from contextlib import ExitStack

import numpy as np

import concourse.bass as bass
import concourse.bacc as bacc
import concourse.mybir as mybir
import concourse.tile as tile
from concourse.bass_utils import run_bass_kernel_spmd

P = 128          # partitions
F = 512          # hidden dim
C4 = F // P      # 4 feature chunks of 128
N = 1024         # tokens
NB = N // P      # 8 token blocks
H = 8            # heads
D = 64           # head dim
B = 8            # batch (one per core)
SCALE = 1.0 / 8.0
CBIAS = 4.5      # exp range shift: E in (0, ~90] inside fp8e4's [2^-10, 240]
EPS = 1e-8

F32 = mybir.dt.float32
BF16 = mybir.dt.bfloat16
FP8 = mybir.dt.float8e4


def _split_free(apnd, group, inner):
    """View a [..., group*inner] stride-1 AP as [..., group, inner]."""
    assert apnd.ap[-1][0] == 1 and apnd.ap[-1][1] == group * inner, apnd.ap
    return bass.AP(
        tensor=apnd.tensor,
        offset=apnd.offset,
        ap=[list(d) for d in apnd.ap[:-1]] + [[inner, group], [1, inner]],
    )


def _ap(t, offset, dims):
    """Raw AP into a tile at an element offset."""
    return bass.AP(tensor=t.tensor, offset=t.offset + offset,
                   ap=[list(d) for d in dims])


def _emit(nc, loop_reps=0):
    xT = nc.dram_tensor("xT", [F, N], BF16, kind="ExternalInput")
    wkT = nc.dram_tensor("wkT", [F, F], BF16, kind="ExternalInput")
    woT = nc.dram_tensor("woT", [F, F], BF16, kind="ExternalInput")
    y = nc.dram_tensor("y", [N, F], F32, kind="ExternalOutput")

    with tile.TileContext(nc) as tc:
        if loop_reps:
            with tc.For_i(0, loop_reps, 1):
                _emit_body(nc, tc, xT, wkT, woT, y)
        else:
            _emit_body(nc, tc, xT, wkT, woT, y)


def _emit_body(nc, tc, xT, wkT, woT, y):
    f32 = F32
    Exp = mybir.ActivationFunctionType.Exp
    Square = mybir.ActivationFunctionType.Square
    Copy = mybir.ActivationFunctionType.Copy
    MULT = mybir.AluOpType.mult
    SUB = mybir.AluOpType.subtract
    ADD = mybir.AluOpType.add
    DR = mybir.MatmulPerfMode.DoubleRow
    AX = mybir.AxisListType.X

    with ExitStack() as ctx:
        persist = ctx.enter_context(tc.tile_pool(name="persist", bufs=1))
        dram = ctx.enter_context(tc.tile_pool(name="dram", bufs=4, space="DRAM"))
        psum = ctx.enter_context(tc.tile_pool(name="psum", bufs=1, space="PSUM"))
        att = ctx.enter_context(tc.tile_pool(name="att", bufs=1))
        attrs = ctx.enter_context(tc.tile_pool(name="attrs", bufs=2))

        # ---- persistent SBUF tiles ----
        xT_sb = persist.tile([P, C4, N], BF16)
        wk_sb = persist.tile([P, C4, F], BF16)
        wo_sb = persist.tile([D, H, F], BF16)
        qstat = persist.tile([D + 1, H, N], BF16)  # q rows + ones row (stat)
        qmov = persist.tile([D + 1, H, N], BF16)   # q rows + w row (moving)
        kqv_bf = persist.tile([P, NB, F], BF16)    # v bf16, [tok, feat]
        v8 = persist.tile([P, NB, F], FP8)         # v fp8
        dv8 = persist.tile([P, NB, F], FP8)        # v - Q(v)
        ones8 = persist.tile([P, 2, D], FP8)       # DoubleRow rowsum stationary
        ones_bf = persist.tile([P, 1], BF16)       # w-matmul stationary
        aT = persist.tile([D, H, N], BF16)         # normalized attn outT
        y_acc = persist.tile([P, NB, F], f32)

        # exp-table prewarm while input DMAs stream
        warm = persist.tile([1, 1], f32)
        nc.vector.memset(warm, 1.0)
        nc.scalar.activation(out=warm, in_=warm, func=Exp, scale=1.0)

        # ---- input DMAs: token-quarters so pair 0 starts early ----
        wk_r = wkT.rearrange("(c p) o -> p c o", p=P)
        nc.gpsimd.dma_start(out=wk_sb[:, 0:2, :], in_=wk_r[:, 0:2, :])
        nc.sync.dma_start(out=wk_sb[:, 2:4, :], in_=wk_r[:, 2:4, :])
        xT_r = xT.rearrange("(c p) n -> p c n", p=P)
        for quar in range(4):
            sl = slice(quar * 256, (quar + 1) * 256)
            eng = nc.sync if quar % 2 == 0 else nc.scalar
            eng.dma_start(out=xT_sb[:, :, sl], in_=xT_r[:, :, sl])
        nc.scalar.dma_start(out=wo_sb, in_=woT.rearrange("(h d) o -> d h o", d=D))

        nc.vector.memset(ones8, 1.0)
        nc.vector.memset(ones_bf, 1.0)
        nc.gpsimd.memset(qstat[D : D + 1, :, :], 1.0)

        def sc_tile():
            return psum.tile([P, N], f32, tag="sc", bufs=2, name="sc")

        # ---- kqv_v pair: token blocks 2p, 2p+1 -> [tok, F] ----
        def emit_kqv_v_pair(p):
            pt = sc_tile()
            for c in range(C4):
                for k in range(2):
                    nc.tensor.matmul(
                        pt[:, k * F : (k + 1) * F],
                        xT_sb[:, c, (2 * p + k) * P : (2 * p + k + 1) * P],
                        wk_sb[:, c, :],
                        start=(c == 0), stop=(c == C4 - 1))
            return pt

        def emit_v_evac(p, pt):
            sl2 = slice(2 * p, 2 * p + 2)
            nc.vector.tensor_copy(out=kqv_bf[:, sl2, :], in_=pt)
            # fp8 v + residual (Pool reads SBUF only)
            nc.gpsimd.tensor_copy(out=v8[:, sl2, :], in_=kqv_bf[:, sl2, :])
            nc.gpsimd.tensor_tensor(
                out=dv8[:, sl2, :], in0=kqv_bf[:, sl2, :], in1=v8[:, sl2, :],
                op=SUB)

        # ---- kqvT chunk ob (heads 2ob, 2ob+1) -> qstat/qmov q rows ----
        def emit_kqvT_chunk(ob):
            pt = sc_tile()
            for c in range(C4):
                for nh in range(2):
                    nc.tensor.matmul(
                        pt[:, nh * F : (nh + 1) * F],
                        wk_sb[:, c, ob * P : (ob + 1) * P],
                        xT_sb[:, c, nh * F : (nh + 1) * F],
                        start=(c == 0), stop=(c == C4 - 1))
            # evac per head into qstat rows 0:64 (DVE handles the base-64
            # move for the odd head); h0 via Scalar (idle pre-exp)
            sqT = attrs.tile([P, N], BF16, tag="sqT", name="sqT")
            nc.scalar.activation(out=sqT, in_=pt, func=Square)
            for hh in range(2):
                h = 2 * ob + hh
                src = pt[hh * D : (hh + 1) * D, :]
                if h == 0:
                    nc.scalar.activation(out=qstat[0:D, h, :], in_=src,
                                         func=Copy)
                else:
                    nc.vector.tensor_copy(out=qstat[0:D, h, :], in_=src)
                eng = nc.sync if h % 2 == 0 else nc.gpsimd
                eng.dma_start(out=qmov[0:D, h, :], in_=qstat[0:D, h, :])
            return sqT

        # ---- w rows for chunk ob: sumsq[n] over each head's 64 features
        # via ones-stationary matmuls on the squared kqvT, then
        # qmov[64, h, :] = 8c - sumsq (Scalar for chunk 0, DVE after) ----
        def emit_wrows(ob, sqT):
            wpt = psum.tile([D, 2, F], f32, tag="acc", name="acc")
            for hh in range(2):
                for nh in range(2):
                    nc.tensor.matmul(
                        wpt[32 * hh : 32 * hh + 1, nh, :],
                        ones_bf[hh * D : (hh + 1) * D, :],
                        sqT[hh * D : (hh + 1) * D, nh * F : (nh + 1) * F],
                        start=True, stop=True,
                        tile_position=(hh * D, 32 * hh))
            for hh in range(2):
                h = 2 * ob + hh
                src_w = wpt[32 * hh : 32 * hh + 1, :, :]
                if ob == 0:
                    nc.scalar.activation(
                        out=qmov[D : D + 1, h, :], in_=src_w,
                        func=Copy, scale=-1.0, bias=8.0 * CBIAS)
                else:
                    nc.vector.tensor_scalar(
                        out=qmov[D : D + 1, h, :], in0=src_w,
                        scalar1=-1.0, scalar2=8.0 * CBIAS, op0=MULT, op1=ADD)

        head_state = {}

        # ---- scores tiles + exp for head h (K=65: q rows + ones*w row) ----
        def emit_scores(h, mbs):
            E = head_state[h]["E"]
            for mb in mbs:
                s = sc_tile()
                for nh in range(2):
                    nc.tensor.matmul(
                        s[:, nh * F : (nh + 1) * F],
                        qstat[:, h, mb * P : (mb + 1) * P],
                        qmov[:, h, nh * F : (nh + 1) * F],
                        start=True, stop=True)
                nc.scalar.activation(out=E[:, mb, :], in_=s, func=Exp,
                                     scale=SCALE)

        def new_head(h):
            head_state[h] = dict(
                E=att.tile([P, NB, N], FP8, tag="E", bufs=3, name="E"))

        # ---- attnv for head h: fp8 DoubleRow; v8+dv8 one group into acc,
        # all-ones stationary into rs (rowsums replicated on 64 rows) ----
        def emit_attnv(h, kts):
            st = head_state[h]
            if "acc" not in st:
                st["acc"] = psum.tile([D, 2, F], f32, tag="acc", name="acc")
                st["rs"] = psum.tile([D, 2, F], f32, tag="rs", name="rs")
            acc, rs = st["acc"], st["rs"]
            E = st["E"]
            for kt in kts:
                for w8, is_one in ((v8, False), (dv8, False), (ones8, True)):
                    for nh in range(2):
                        out = (rs if is_one else acc)[:, nh, :]
                        lhsT = (w8[:, :, :] if is_one
                                else w8[:, 2 * kt : 2 * kt + 2,
                                        h * D : (h + 1) * D])
                        nc.tensor.matmul(
                            out, lhsT,
                            E[:, 2 * kt : 2 * kt + 2, nh * F : (nh + 1) * F],
                            start=(kt == kts[0] if is_one
                                   else (kt == 0 and w8 is v8)),
                            stop=(kt == kts[-1] if is_one
                                  else (kt == 3 and w8 is dv8)),
                            perf_mode=DR)

        # ---- reciprocal of rowsums: rs psum -> SBUF ----
        def emit_recip(h):
            st = head_state[h]
            rs_sb = attrs.tile([D, N], f32, tag="rs_sb", name="rs_sb")
            nc.vector.reciprocal_approx_fast(
                out=_split_free(rs_sb[:, :], 2, F), in_=st["rs"][:, :, :])
            st["rs_sb"] = rs_sb

        # ---- evac: aT[, h, ] = acc * recip_rowsums ----
        def emit_evac(h):
            st = head_state[h]
            nc.vector.tensor_tensor(
                out=_split_free(aT[:, h, :], 2, F),
                in0=st["acc"][:, :, :],
                in1=_split_free(st["rs_sb"][:, :], 2, F),
                op=MULT)

        # ---- yproj partial for head pair pc over 2-block tiles ----
        def emit_ypartial(pc, nbs, heads=(0, 1), with_dma=False, add=None):
            first_pair = (not add) if add is not None else (pc == 0)
            for i in range(0, len(nbs), 2):
                pt = sc_tile()
                for hh in heads:
                    h = 2 * pc + hh
                    for k in range(2):
                        nb = nbs[i + k]
                        nc.tensor.matmul(
                            pt[:, k * F : (k + 1) * F],
                            aT[:, h, nb * P : (nb + 1) * P],
                            wo_sb[:, h, :],
                            start=(hh == heads[0]), stop=(hh == heads[-1]))
                nb0 = nbs[i]
                if first_pair:
                    nc.vector.tensor_copy(out=y_acc[:, nb0 : nb0 + 2, :], in_=pt)
                else:
                    nc.vector.tensor_add(out=y_acc[:, nb0 : nb0 + 2, :],
                                         in0=y_acc[:, nb0 : nb0 + 2, :], in1=pt)
                if with_dma:
                    eng = nc.sync if nb0 % 4 == 0 else nc.gpsimd
                    eng.dma_start(
                        out=y.rearrange("(b p) f -> p b f", p=P)[:, nb0 : nb0 + 2, :],
                        in_=y_acc[:, nb0 : nb0 + 2, :])

        # ========== prologue: projections, w rows, heads 0-1 scores ==========
        for p in range(4):
            emit_v_evac(p, emit_kqv_v_pair(p))
        new_head(0)
        sqT0 = emit_kqvT_chunk(0)
        sqT1 = emit_kqvT_chunk(1)
        emit_wrows(0, sqT0)
        emit_scores(0, [0, 1])
        emit_wrows(1, sqT1)
        emit_scores(0, [2, 3])
        sqT2 = emit_kqvT_chunk(2)
        emit_scores(0, [4, 5])
        emit_wrows(2, sqT2)
        sqT3 = emit_kqvT_chunk(3)
        emit_scores(0, [6, 7])
        emit_wrows(3, sqT3)
        new_head(1)
        emit_scores(1, [0, 1, 2, 3])
        emit_scores(1, [4, 5, 6, 7])

        # ========== phase 2: catch up attnv(0) and attnv(1) ==========
        new_head(2)
        emit_scores(2, [0, 1])
        emit_attnv(0, [0, 1, 2, 3])
        emit_scores(2, [2, 3])
        emit_recip(0)
        emit_evac(0)
        emit_scores(2, [4, 5])
        emit_attnv(1, [0, 1, 2, 3])
        emit_scores(2, [6, 7])
        emit_recip(1)
        emit_evac(1)

        # ========== steady phases 3-7 ==========
        for h in range(3, H):
            new_head(h)
            emit_scores(h, [0, 1])
            emit_attnv(h - 1, [0, 1, 2, 3])
            emit_scores(h, [2, 3])
            emit_recip(h - 1)
            # filler batches (ypartial pc needs evac(2pc+1))
            if h == 3:
                emit_ypartial(0, [0, 1, 2, 3])
            elif h == 4:
                emit_ypartial(0, [4, 5, 6, 7])
            elif h == 5:
                emit_ypartial(1, [0, 1, 2, 3])
            elif h == 6:
                emit_ypartial(1, [4, 5, 6, 7])
            elif h == 7:
                emit_ypartial(2, list(range(NB)))
            emit_scores(h, [4, 5])
            emit_evac(h - 1)
            emit_scores(h, [6, 7])

        # ========== tail ==========
        h = H - 1
        emit_attnv(h, [0, 1, 2, 3])
        emit_recip(h)
        # h6 half of the final pair while h7's recip/evac run
        emit_ypartial(3, list(range(NB)), heads=(0,), add=True)
        emit_evac(h)
        emit_ypartial(3, list(range(NB)), heads=(1,), add=True, with_dma=True)


_NC_CACHE = None


def build_nc():
    global _NC_CACHE
    if _NC_CACHE is None:
        nc = bacc.Bacc(trn_type="TRN2")
        _emit(nc)
        nc.finalize()
        _NC_CACHE = nc
    return _NC_CACHE


def _eff_weightT(weight, style):
    """Host: modulated+demodulated weight, transposed, bf16.
    weight [O, I] fp32, style [I] fp32 -> [I, O] bf16."""
    import ml_dtypes
    w = weight * style[None, :]
    w = w * (1.0 / np.sqrt((w * w).sum(axis=1) + EPS))[:, None]
    return np.ascontiguousarray(w.T.astype(ml_dtypes.bfloat16))


def make_in_maps(x, s, k_weight, k_aff_w, k_aff_b, o_weight, o_aff_w, o_aff_b):
    import ml_dtypes
    f = np.float32
    bf = ml_dtypes.bfloat16
    x = np.asarray(x, f)
    s = np.asarray(s, f)
    k_weight = np.asarray(k_weight, f)
    k_aff_w = np.asarray(k_aff_w, f)
    k_aff_b = np.asarray(k_aff_b, f)
    o_weight = np.asarray(o_weight, f)
    o_aff_w = np.asarray(o_aff_w, f)
    o_aff_b = np.asarray(o_aff_b, f)
    in_maps = []
    for b in range(B):
        style_k = s[b] @ k_aff_w.T + k_aff_b
        style_o = s[b] @ o_aff_w.T + o_aff_b
        in_maps.append({
            "xT": np.ascontiguousarray(x[b].T.astype(bf)),
            "wkT": _eff_weightT(k_weight, style_k),
            "woT": _eff_weightT(o_weight, style_o),
        })
    return in_maps


def kernel(x, s, k_weight, k_aff_w, k_aff_b, o_weight, o_aff_w, o_aff_b):
    assert x.shape == (B, N, F), x.shape
    nc = build_nc()
    in_maps = make_in_maps(x, s, k_weight, k_aff_w, k_aff_b,
                           o_weight, o_aff_w, o_aff_b)
    res = run_bass_kernel_spmd(nc, in_maps, list(range(B)))
    return np.stack([res.results[b]["y"] for b in range(B)], axis=0)
